# revision 1
# baseline (speedup 1.0000x reference)
"""Trainium2 Bass kernel for nn_BDHModel (scatter_memory).

Computes, for T tokens:
  raw  = projection[tokens]                  # [T, N] gather
  thr  = 20th largest per row; acts = raw >= thr   (binary, K=20 active)
  scan: pred = sigma @ x; tension_t = 1 - <pred,x>/(|pred||x|+1e-8);
        sigma += 0.01 * outer(x,x), clipped to [0,1]

Key algebraic identity used on device: sigma starts at 0 and each entry grows
by +0.01 per co-activation. The clip at 1.0 binds only if some neuron pair
co-activates >100 times; for K=20-sparse random activations over T=256 steps
the max co-activation count is ~20 (verified host-side; numpy fallback
otherwise). With clip never binding:

  sigma_t = 0.01 * X_{<t}^T X_{<t}        (X = binary acts [T,N])
  pred_t  = 0.01 * X_{<t}^T g_t,  g_t = X_{<t} x_t = G[:t, t],  G = X X^T
  <pred_t, x_t>  = 0.01 * sum_{s<t} G[s,t]^2
  |pred_t|^2     = 1e-4 * g_t^T G_{<t,<t} g_t = 1e-4 * sum_s L[s,t] (G L)[s,t]
  with L = strictly-"earlier" masked G. So the serial scan collapses into a
  few small matmuls on the token-gram matrix G [T,T].

Device pipeline (single-core program, replicated SPMD on 8 cores):
  1. dma_gather of the T projection rows (token ids baked at compile time;
     the int16 index limit is handled by splitting the vocab at 32768 and
     permuting tokens so low-vocab tokens occupy a slot prefix; the one
     mixed 128-token chunk is assembled via a parallel scratch gather and a
     partition-aligned stitch copy).
  2. Exact top-20 threshold per 1024-wide row on the DVE:
     - segmented path (validity host-verified per input): top-8 of each of
       16 64-wide segments via max8, then 3 max8 + 2 match_replace merge
       rounds over the 128 candidates; thr = 4th value of round 3.
     - fallback: 3 max8 + 2 match_replace rounds over the full row.
  3. acts = (raw >= thr) as bf16 (with per-row count via accum_out);
     PE-transpose to neuron-major XT.
  4. G = XT^T XT (PE, bf16 exact: entries are ints <= 20).
  5. L = G * mask, mask[s,t] = [time(s) < time(t)] precomputed host-side
     (handles the vocab-split token permutation).
  6. M = G @ L (PE); dot = colsum(L*L); pn2 = colsum(L*M).
  7. tension = 1 - dot / (sqrt(pn2*cnt) + 1e-6)   [identical regrouping of
     the reference's 1 - 0.01*dot / (0.01*sqrt(pn2)*sqrt(cnt) + 1e-8)].
  8. DMA out [1, T]; host un-permutes slots back to time order.
"""

import os
import numpy as np

T, N, K = 256, 1024, 20
VOCAB, HALF = 50257, 32768
NCH = N // 128   # 8 neuron chunks
TCH = T // 128   # 2 token chunks

LAST_RESULT = None  # BassKernelResults of the most recent device run


def _numpy_fallback(projection, sigma, tokens, plasticity):
    """Exact step-by-step emulation of the reference (f32). Only used if the
    fast-path preconditions fail (never, for the reference input family)."""
    proj = np.asarray(projection, np.float32)
    raw = proj[np.asarray(tokens)]
    kth = np.partition(raw, N - K, axis=1)[:, N - K]
    acts = (raw >= kth[:, None]).astype(np.float32)
    sig = np.array(sigma, np.float32, copy=True)
    out = np.zeros(T, np.float32)
    for t in range(T):
        x = acts[t]
        pred = (sig @ x).astype(np.float32)
        pn2 = np.float32(np.dot(pred, pred))
        pn = np.sqrt(pn2 if pn2 > 0 else np.float32(1.0))
        xn = np.float32(np.sqrt(np.dot(x, x)))
        overlap = np.float32(np.dot(pred, x)) / (pn * xn + np.float32(1e-8))
        out[t] = np.float32(1.0) - overlap if pn2 > 0 else np.float32(1.0)
        if plasticity:
            sig = np.clip(sig + np.float32(0.01) * np.outer(x, x), 0.0, 1.0)
    return out


def _plan_gathers(ptok, nlow):
    """Returns (gathers, stitches). Each gather: (dest, chunk, half, idxs)
    with dest in {"raw", "scr"}; all gathers write disjoint tiles and run in
    parallel. Each stitch: (chunk, part_off, rows) — a partition-aligned ACT
    copy scr[part_off:part_off+rows] -> raw_chunk[part_off:...]."""
    gathers, stitches = [], []
    for c in range(TCH):
        lc = int(np.clip(nlow - 128 * c, 0, 128))
        hc = 128 - lc
        lo = ptok[128 * c: 128 * c + lc]
        hi = ptok[128 * c + lc: 128 * (c + 1)]
        if hc == 0:
            gathers.append(("raw", c, 0, lo))
        elif lc == 0:
            gathers.append(("raw", c, 1, hi - HALF))
        else:
            # lows go to scratch partitions [0, lc); highs go straight into
            # the chunk with an lc-row junk prefix (overwritten by the
            # stitch copy, which starts at partition 0 as engines require)
            gathers.append(("scr", c, 0, lo))
            idxs = np.concatenate([np.zeros(lc, np.int64), hi - HALF])
            gathers.append(("raw", c, 1, idxs))
            stitches.append((c, 0, lc))
    return gathers, stitches


def _wrap_idxs(idxs):
    """dma_gather index layout: slot j -> row j%16, col j//16, replicated to
    128 partitions; 8 int16 columns per gather."""
    w = np.full((16, 8), -1, np.int16)
    for j, v in enumerate(idxs):
        w[j % 16, j // 16] = v
    return np.tile(w, (8, 1))


def _build(tokens_np, nseg=16):
    """Build the Bass module with token ids baked in. Returns (nc, in_map, perm)."""
    from contextlib import ExitStack
    import concourse.bacc as bacc
    import concourse.mybir as mybir
    import concourse.tile as tile
    from concourse import masks
    from concourse.tile import add_dep_helper

    dt = mybir.dt
    Alu = mybir.AluOpType
    Act = mybir.ActivationFunctionType

    tok = np.asarray(tokens_np, np.int64)
    lows = np.where(tok < HALF)[0]
    highs = np.where(tok >= HALF)[0]
    perm = np.concatenate([lows, highs])      # slot -> original position
    ptok = tok[perm]
    nlow = len(lows)
    gathers, stitches = _plan_gathers(ptok, nlow)

    gidx_np = np.concatenate([_wrap_idxs(g[3]) for g in gathers], axis=1)
    tv = perm.astype(np.float32)              # original time per slot
    # msk[m][p, t]  = 1.0 iff time(128m+p) < time(t)   (L in [s, t] layout)
    # msk2[m][p, s] = 1.0 iff time(s) < time(128m+p)   (L^T in [t, s] layout)
    msk_np = np.concatenate(
        [(tv[None, :] > tv[128 * m: 128 * (m + 1), None]).astype(np.float32)
         for m in range(TCH)], axis=1)        # [128, TCH*T]
    msk2_np = np.concatenate(
        [(tv[None, :] < tv[128 * m: 128 * (m + 1), None]).astype(np.float32)
         for m in range(TCH)], axis=1)        # [128, TCH*T]

    nc = bacc.Bacc("TRN2", target_bir_lowering=False, debug=False,
                   enable_asserts=False, num_devices=1)

    proj_d = nc.dram_tensor("proj", [VOCAB, N], dt.float32, kind="ExternalInput")
    gidx_d = nc.dram_tensor("gidx", list(gidx_np.shape), dt.int16, kind="ExternalInput")
    msk_d = nc.dram_tensor("msk", [128, TCH * T], dt.float32, kind="ExternalInput")
    msk2_d = nc.dram_tensor("msk2", [128, TCH * T], dt.float32, kind="ExternalInput")
    out_d = nc.dram_tensor("tens", [128, TCH], dt.float32, kind="ExternalOutput")

    with tile.TileContext(nc) as tc, ExitStack() as ctx:
        pool = ctx.enter_context(tc.tile_pool(name="main", bufs=1))
        ppt = ctx.enter_context(tc.tile_pool(name="ppt", bufs=4, space="PSUM"))
        pacc = ctx.enter_context(tc.tile_pool(name="pacc", bufs=1, space="PSUM"))

        raw = pool.tile([128, TCH * N], dt.float32, tag="raw")
        scr = pool.tile([128, N], dt.float32, tag="scr")
        gidx = pool.tile([128, gidx_np.shape[1]], dt.int16, tag="gidx")
        msk = pool.tile([128, TCH * T], dt.float32, tag="msk")
        msk2 = pool.tile([128, TCH * T], dt.float32, tag="msk2")
        seg_topk = nseg > 0
        cand = pool.tile([128, 8 * max(nseg, 1) * TCH], dt.float32, tag="cand")
        rawc = None if seg_topk else pool.tile([128, TCH * N], dt.float32, tag="rawc")
        m8 = pool.tile([128, 24 * TCH], dt.float32, tag="m8")
        acts = pool.tile([128, TCH * N], dt.bfloat16, tag="acts")
        ident = pool.tile([128, 128], dt.bfloat16, tag="ident")
        xt = pool.tile([128, NCH * T], dt.bfloat16, tag="xt")
        gb = pool.tile([128, TCH * T], dt.bfloat16, tag="gb")
        lt = pool.tile([128, TCH * T], dt.float32, tag="lt")
        lb = pool.tile([128, TCH * T], dt.bfloat16, tag="lb")
        dump = pool.tile([128, T], dt.float32, tag="dump")
        prod1 = pool.tile([128, TCH * T], dt.float32, tag="prod1")
        prod2 = pool.tile([128, TCH * T], dt.float32, tag="prod2")
        cnt_pm = pool.tile([128, TCH], dt.float32, tag="cnt_pm")
        dotv = pool.tile([128, TCH], dt.float32, tag="dotv")
        pn2v = pool.tile([128, TCH], dt.float32, tag="pn2v")
        q_v = pool.tile([128, TCH], dt.float32, tag="q_v")
        r_v = pool.tile([128, TCH], dt.float32, tag="r_v")
        rec_v = pool.tile([128, TCH], dt.float32, tag="rec_v")
        prod_v = pool.tile([128, TCH], dt.float32, tag="prod_v")
        tens_v = pool.tile([128, TCH], dt.float32, tag="tens_v")
        pre_v = pool.tile([128, 1], dt.float32, tag="pre_v")

        # --- constants, ACT table preloads, small input DMAs ---
        nc.sync.dma_start(gidx[:], gidx_d.ap())
        nc.sync.dma_start(msk[:], msk_d.ap())
        nc.sync.dma_start(msk2[:], msk2_d.ap())
        # preload ACT function tables off the critical path (sqrt(1)=1)
        nc.gpsimd.memset(pre_v[:], 1.0)
        nc.scalar.activation(pre_v[:], pre_v[:], Act.Copy)
        nc.scalar.activation(pre_v[:], pre_v[:], Act.Sqrt)
        masks.make_identity(nc, ident[:])

        # --- 1. gathers (all parallel; disjoint dest tiles) + stitch ---
        raw3 = raw[:].rearrange("p (c n) -> p c n", n=N)
        scr3 = scr[:].rearrange("p (c n) -> p c n", n=N)
        proj_ap = proj_d.ap()
        for g, (dest, c, half, idxs) in enumerate(gathers):
            out_ap = raw3[:, c: c + 1, :] if dest == "raw" else scr3[:, 0:1, :]
            nc.gpsimd.dma_gather(
                out_ap=out_ap,
                in_ap=proj_ap[HALF:, :] if half else proj_ap,
                idxs_ap=gidx[:, 8 * g: 8 * g + (len(idxs) + 15) // 16],
                num_idxs=len(idxs),
                num_idxs_reg=int(len(idxs)),
                elem_size=N,
            )
        for c, off, rows in stitches:
            nc.scalar.activation(
                raw[off:off + rows, c * N:(c + 1) * N],
                scr[off:off + rows, :], Act.Copy)

        # --- 2+3. per token-chunk: top-20 threshold, acts (+ row counts) ---
        prev_last = None
        for c in range(TCH):
            rc = raw[:, c * N:(c + 1) * N]
            chunk_ops = []
            if seg_topk:
                segw = N // nseg
                cd = cand[:, c * 8 * nseg:(c + 1) * 8 * nseg]
                for s in range(nseg):
                    op = nc.vector.max(
                        cd[:, s * 8:(s + 1) * 8],
                        rc[:, s * segw:(s + 1) * segw])
                    chunk_ops.append(op)
                sel = cd
            else:
                op = nc.scalar.activation(rawc[:, c * N:(c + 1) * N], rc, Act.Copy)
                sel = rawc[:, c * N:(c + 1) * N]
                rc = sel
                chunk_ops.append(op)
            m1 = m8[:, c * 24 + 0: c * 24 + 8]
            m2 = m8[:, c * 24 + 8: c * 24 + 16]
            m3 = m8[:, c * 24 + 16: c * 24 + 24]
            src = sel if seg_topk else raw[:, c * N:(c + 1) * N]
            chunk_ops.append(nc.vector.max(m1, src))
            chunk_ops.append(nc.vector.match_replace(src, m1, src, -1e30))
            chunk_ops.append(nc.vector.max(m2, src))
            chunk_ops.append(nc.vector.match_replace(src, m2, src, -1e30))
            chunk_ops.append(nc.vector.max(m3, src))
            thr = m8[:, c * 24 + 19: c * 24 + 20]   # 4th of round 3 = 20th
            last = nc.vector.tensor_scalar(
                acts[:, c * N:(c + 1) * N], rc, thr, None, Alu.is_ge,
                Alu.add, accum_out=cnt_pm[:, c: c + 1])
            chunk_ops.append(last)
            # keep the DVE chain chunk-ordered so chunk 0 finishes early and
            # its transposes/G overlap chunk 1's top-k
            if prev_last is not None:
                for op in chunk_ops:
                    add_dep_helper(op.ins, prev_last.ins, sync=False,
                                   reason="chunk-order DVE chain")
            prev_last = last

        # --- 3b. PE transpose acts -> XT [neuron, token] (bf16) ---
        # blocks grouped by token-half r so all r=0 work (transpose, copy,
        # and the G half-matmuls below) overlaps chunk 1's top-k; four
        # 128x128 transposes pack into one PSUM tile so one wide copy
        # evacuates them. xt free layout: index = r*N + cn*128.
        for r in range(TCH):
            for g in range(NCH // 4):
                pt = ppt.tile([128, 512], dt.bfloat16, tag="pt")
                for j in range(4):
                    cn = g * 4 + j
                    nc.tensor.transpose(
                        pt[:, j * 128:(j + 1) * 128],
                        acts[:, r * N + cn * 128: r * N + (cn + 1) * 128],
                        ident[:],
                    )
                dst = xt[:, r * N + g * 512: r * N + (g + 1) * 512]
                if r == 0 or g % 2 == 0:
                    # ACT: the DVE must not be interrupted mid-top-k (r=0)
                    nc.scalar.activation(dst, pt[:], Act.Copy)
                else:
                    nc.vector.tensor_copy(dst, pt[:])

        # --- 4. G = X X^T  [T, T] f32 psum, via bf16 matmuls (exact),
        #        split by token-half r so the r=0 half runs early ---
        gps = []
        for m in range(TCH):
            gp = pacc.tile([128, T], dt.float32, tag=f"g{m}")
            gps.append(gp)
        # m-outer: gps[0] completes first so the DVE's masked muls (below)
        # start while gps[1]'s groups are still on the PE
        for m in range(TCH):
            for r in range(TCH):
                for cn in range(NCH):
                    nc.tensor.matmul(
                        gps[m][:, r * 128:(r + 1) * 128],
                        xt[:, m * N + cn * 128: m * N + (cn + 1) * 128],
                        xt[:, r * N + cn * 128: r * N + (cn + 1) * 128],
                        start=(cn == 0), stop=(cn == NCH - 1),
                    )

        # --- 5+6. masked prefix matrices straight from PSUM, M^T = L^T G,
        #        and the dot/pn2 row reductions — all split by token-half so
        #        every piece gated only on r=0 data runs during chunk 1's
        #        top-k. Emission order == dependency order (r ascending).
        #        lb = bf16(G * msk)   (L, [s, t] layout — lhsT for M^T)
        #        lt = f32 (G * msk2)  (L^T, [t, s] layout — for row TTRs)
        #        gb = bf16(G)         (rhs for M^T) ---
        mts = []
        for m in range(TCH):
            mt = pacc.tile([128, T], dt.float32, tag=f"mt{m}")
            mts.append(mt)

        # gb halves by r (ACT — free during chunk 1's top-k); lb/lt as full
        # DVE ops (DVE is the serial resource; splitting only adds overhead)
        for r in range(TCH):
            for m in range(TCH):
                sl = slice(m * T + r * 128, m * T + (r + 1) * 128)
                nc.scalar.activation(gb[:, sl],
                                     gps[m][:, r * 128:(r + 1) * 128], Act.Copy)
        # NOTE: tensor_tensor_reduce is rejected by this runtime (device
        # NRT_EXEC_UNIT_UNRECOVERABLE) — reductions use an exact DVE product
        # followed by an ACT Copy with accum_out (HW-verified) instead.
        # Per-block interleave: all m=0 work is emitted before anything
        # gated on gps[1], so the DVE isn't head-of-line blocked.
        for m in range(TCH):
            ltm = lt[:, m * T:(m + 1) * T]
            nc.vector.tensor_mul(lb[:, m * T:(m + 1) * T], gps[m][:],
                                 msk[:, m * T:(m + 1) * T])
            nc.vector.tensor_mul(ltm, gps[m][:], msk2[:, m * T:(m + 1) * T])
            # dot[t] = sum_s L^T[t,s]^2 — off the critical DVE sequence
            # (GPSIMD product; dot only gates the final subtract)
            nc.gpsimd.tensor_mul(prod1[:, m * T:(m + 1) * T], ltm, ltm)
            nc.scalar.activation(dump[:], prod1[:, m * T:(m + 1) * T],
                                 Act.Copy, accum_out=dotv[:, m: m + 1])
        for m in range(TCH):
            for b in range(TCH):
                nc.tensor.matmul(
                    mts[m][:],
                    lb[:, b * T + m * 128: b * T + (m + 1) * 128],
                    gb[:, b * T:(b + 1) * T],
                    start=(b == 0), stop=(b == TCH - 1),
                )
            nc.vector.tensor_mul(prod2[:, m * T:(m + 1) * T],
                                 lt[:, m * T:(m + 1) * T], mts[m][:])
            # scale = cnt folds q = pn2*cnt into the accumulate (exact: all
            # terms are integers < 2^24), so sqrt follows directly on ACT
            nc.scalar.activation(dump[:], prod2[:, m * T:(m + 1) * T],
                                 Act.Copy, scale=cnt_pm[:, m: m + 1],
                                 accum_out=q_v[:, m: m + 1])

        # --- 7. final per-token math on [128, TCH] (token-major):
        #     tension = 1 - dot/denom = (denom - dot)/denom,
        #     denom = sqrt(pn2*cnt) + 1e-6; q = pn2*cnt from the accum above.
        # Split per token-block column: block 0's chain runs while block 1's
        # pn2 accumulate is still in flight. ---
        for m in range(TCH):
            sl = slice(m, m + 1)
            nc.scalar.activation(r_v[:, sl], q_v[:, sl], Act.Sqrt)
            nc.vector.tensor_scalar_add(r_v[:, sl], r_v[:, sl], 1e-6)
            nc.vector.tensor_tensor(prod_v[:, sl], r_v[:, sl], dotv[:, sl],
                                    Alu.subtract)
            nc.vector.reciprocal(rec_v[:, sl], r_v[:, sl])
            nc.vector.tensor_mul(tens_v[:, sl], prod_v[:, sl], rec_v[:, sl])

        # --- 8. output: plain [128, TCH] DMA; host maps (p, c) -> t = 128c+p ---
        nc.sync.dma_start(out_d.ap(), tens_v[:])

    nc.compile()

    in_map = {
        "proj": None,  # filled by caller (f32 [VOCAB, N])
        "gidx": gidx_np,
        "msk": msk_np,
        "msk2": msk2_np,
    }
    return nc, in_map, perm


def _check_input(projection, sigma, tokens):
    """Host-side guards. Returns (fast_ok, nseg):
    fast_ok — the algebraic rewrite is exact (sigma==0, clip never binds);
    nseg    — widest valid segmentation for the segmented top-k (a
    segmentation is valid when taking the top-8 of every segment still
    captures all of each row's top-20 values), or 0 for the full-row path."""
    if np.any(np.asarray(sigma)):
        return False, 0
    proj = np.asarray(projection, np.float32)
    raw = proj[np.asarray(tokens)]
    kth = np.partition(raw, N - K, axis=1)[:, N - K]
    acts = (raw >= kth[:, None]).astype(np.float32)
    coact = acts.T @ acts
    fast_ok = float(coact.max()) <= 100.0
    nseg = 0
    for cand_nseg in (8, 16):
        segs = raw.reshape(T, cand_nseg, N // cand_nseg)
        cand = -np.sort(-segs, axis=2)[:, :, :8].reshape(T, cand_nseg * 8)
        thr_dev = -np.sort(-cand, axis=1)[:, K - 1]
        if bool(np.all(thr_dev == kth)):
            nseg = cand_nseg
            break
    return fast_ok, nseg


def kernel(projection, sigma, tokens, plasticity):
    global LAST_RESULT
    projection = np.ascontiguousarray(np.asarray(projection, np.float32))
    sigma = np.asarray(sigma, np.float32)
    tokens = np.asarray(tokens).astype(np.int64)
    plast = int(np.asarray(plasticity).reshape(-1)[0]) if np.ndim(plasticity) else int(plasticity)

    if not plast:
        # sigma never updates; with sigma == 0, pred == 0 -> tension == 1.
        if not np.any(sigma):
            return np.ones(T, np.float32)
        return _numpy_fallback(projection, sigma, tokens, plast)
    fast_ok, nseg = _check_input(projection, sigma, tokens)
    if not fast_ok:
        return _numpy_fallback(projection, sigma, tokens, plast)

    from concourse.bass_utils import run_bass_kernel_spmd

    nc, in_map, perm = _build(tokens, nseg=nseg)
    in_map["proj"] = projection
    n_cores = int(os.environ.get("BDH_CORES", "8"))
    try:
        res = run_bass_kernel_spmd(
            nc,
            [dict(in_map) for _ in range(n_cores)],
            core_ids=list(range(n_cores)),
        )
    except ModuleNotFoundError:
        # BASS_TRACE was requested but this axon build has no NTFF hook.
        os.environ["BASS_NEVER_TRACE"] = "1"
        res = run_bass_kernel_spmd(
            nc,
            [dict(in_map) for _ in range(n_cores)],
            core_ids=list(range(n_cores)),
        )
    LAST_RESULT = res
    # device layout [p, c] -> slot t = 128c + p; then slot -> original time
    tens_slots = res.results[0]["tens"].reshape(128, TCH).T.reshape(T)
    out = np.empty(T, np.float32)
    out[perm] = tens_slots.astype(np.float32)
    return out



# revision 13
# speedup vs baseline: 1.0657x; 1.0657x over previous
"""Trainium2 Bass kernel for nn_BDHModel (scatter_memory).

Computes, for T tokens:
  raw  = projection[tokens]                  # [T, N] gather
  thr  = 20th largest per row; acts = raw >= thr   (binary, K=20 active)
  scan: pred = sigma @ x; tension_t = 1 - <pred,x>/(|pred||x|+1e-8);
        sigma += 0.01 * outer(x,x), clipped to [0,1]

Key algebraic identity used on device: sigma starts at 0 and each entry grows
by +0.01 per co-activation. The clip at 1.0 binds only if some neuron pair
co-activates >100 times; for K=20-sparse random activations over T=256 steps
the max co-activation count is ~20 (verified host-side; numpy fallback
otherwise). With clip never binding:

  sigma_t = 0.01 * X_{<t}^T X_{<t}        (X = binary acts [T,N])
  pred_t  = 0.01 * X_{<t}^T g_t,  g_t = X_{<t} x_t = G[:t, t],  G = X X^T
  <pred_t, x_t>  = 0.01 * sum_{s<t} G[s,t]^2
  |pred_t|^2     = 1e-4 * g_t^T G_{<t,<t} g_t = 1e-4 * sum_s L[s,t] (G L)[s,t]
  with L = strictly-"earlier" masked G. So the serial scan collapses into a
  few small matmuls on the token-gram matrix G [T,T].

Device pipeline (single-core program, replicated SPMD on 8 cores):
  1. dma_gather of the T projection rows (token ids baked at compile time;
     the int16 index limit is handled by splitting the vocab at 32768 and
     permuting tokens so low-vocab tokens occupy a slot prefix; the one
     mixed 128-token chunk is assembled via a parallel scratch gather and a
     partition-aligned stitch copy).
  2. Exact top-20 threshold per 1024-wide row on the DVE:
     - segmented path (validity host-verified per input): top-8 of each of
       nseg segments via max8, then 3 max8 + 2 match_replace merge
       rounds over the candidates; thr = 4th value of round 3.
     - fallback: 3 max8 + 2 match_replace rounds over the full row.
  3. acts = (raw >= thr) as bf16 with per-row count via accum_out. Chunk 1's
     compare is split into two 512-wide halves so its PE transposes can
     start on the first half while the second is still comparing.
  4. PE-transpose acts -> neuron-major XT; chunk-0 transposes evacuate
     PSUM via ACT (DVE is mid-top-k), chunk-1 via DVE (idle then).
  5. G = XT^T XT (PE, bf16 exact: entries are ints <= 20). The r=1 matmul
     groups are split cn 0..3 / 4..7 so they start after the first
     PSUM-evacuation half.
  6. Tail, all fused:
       gb  = bf16(G)            (ACT copies; rhs for M^T, input for lt)
       lb  = bf16(G * msk)      (DVE; lhsT for M^T)    [s, t] layout
       lt  = bf16(gb * msk2b)   (DVE, all-SBUF bf16)   [t, s] layout
       dot[t] = sum_s lt^2      (GPSIMD scalar_tensor_tensor accum_out)
       mts = L^T G              (PE)
       q[t] = cnt * sum_s lt * mts   (DVE stt, scalar=cnt, accum_out)
       r = sqrt(q + 1e-12)      (ACT, bias AP; == sqrt(q)+1e-6 at q=0)
       tension = 1 - dot / r    (DVE reciprocal + 2 fused ops)
  7. Output via dma_scatter_add prepare/trigger: descriptors are generated
     on GPSIMD early (idle mid-kernel); after the final value is ready only
     the trigger + transfer + completion-sem latency remains. The DRAM
     output is [128, 64] f32 (256B-row scatter constraint; host reads
     cols 0..1); run_bass_kernel_spmd zero-fills outputs so the add is a
     plain write.
  8. Host un-permutes slots back to time order.
"""

import os
import numpy as np

T, N, K = 256, 1024, 20
VOCAB, HALF = 50257, 32768
NCH = N // 128   # 8 neuron chunks
TCH = T // 128   # 2 token chunks
OUTW = 64        # scatter-add rows must be 256B multiples -> 64 f32 cols

LAST_RESULT = None  # BassKernelResults of the most recent device run


def _numpy_fallback(projection, sigma, tokens, plasticity):
    """Exact step-by-step emulation of the reference (f32). Only used if the
    fast-path preconditions fail (never, for the reference input family)."""
    proj = np.asarray(projection, np.float32)
    raw = proj[np.asarray(tokens)]
    kth = np.partition(raw, N - K, axis=1)[:, N - K]
    acts = (raw >= kth[:, None]).astype(np.float32)
    sig = np.array(sigma, np.float32, copy=True)
    out = np.zeros(T, np.float32)
    for t in range(T):
        x = acts[t]
        pred = (sig @ x).astype(np.float32)
        pn2 = np.float32(np.dot(pred, pred))
        pn = np.sqrt(pn2 if pn2 > 0 else np.float32(1.0))
        xn = np.float32(np.sqrt(np.dot(x, x)))
        overlap = np.float32(np.dot(pred, x)) / (pn * xn + np.float32(1e-8))
        out[t] = np.float32(1.0) - overlap if pn2 > 0 else np.float32(1.0)
        if plasticity:
            sig = np.clip(sig + np.float32(0.01) * np.outer(x, x), 0.0, 1.0)
    return out


def _plan_gathers(ptok, nlow):
    """Returns (gathers, stitches). Each gather: (dest, chunk, half, idxs)
    with dest in {"raw", "scr"}; all gathers write disjoint tiles and run in
    parallel. Each stitch: (chunk, part_off, rows) — a partition-aligned ACT
    copy scr[part_off:part_off+rows] -> raw_chunk[part_off:...]."""
    gathers, stitches = [], []
    for c in range(TCH):
        lc = int(np.clip(nlow - 128 * c, 0, 128))
        hc = 128 - lc
        lo = ptok[128 * c: 128 * c + lc]
        hi = ptok[128 * c + lc: 128 * (c + 1)]
        if hc == 0:
            gathers.append(("raw", c, 0, lo))
        elif lc == 0:
            gathers.append(("raw", c, 1, hi - HALF))
        else:
            # lows go to scratch partitions [0, lc); highs go straight into
            # the chunk with an lc-row junk prefix (overwritten by the
            # stitch copy, which starts at partition 0 as engines require)
            gathers.append(("scr", c, 0, lo))
            idxs = np.concatenate([np.zeros(lc, np.int64), hi - HALF])
            gathers.append(("raw", c, 1, idxs))
            stitches.append((c, 0, lc))
    return gathers, stitches


def _wrap_idxs(idxs):
    """dma_gather index layout: slot j -> row j%16, col j//16, replicated to
    128 partitions; 8 int16 columns per gather."""
    w = np.full((16, 8), -1, np.int16)
    for j, v in enumerate(idxs):
        w[j % 16, j // 16] = v
    return np.tile(w, (8, 1))


def _build(tokens_np, nseg=8):
    """Build the Bass module with token ids baked in. Returns (nc, in_map, perm)."""
    from contextlib import ExitStack
    import concourse.bacc as bacc
    import concourse.mybir as mybir
    import concourse.tile as tile
    from concourse import masks
    from concourse.tile import add_dep_helper

    dt = mybir.dt
    Alu = mybir.AluOpType
    Act = mybir.ActivationFunctionType

    tok = np.asarray(tokens_np, np.int64)
    lows = np.where(tok < HALF)[0]
    highs = np.where(tok >= HALF)[0]
    perm = np.concatenate([lows, highs])      # slot -> original position
    ptok = tok[perm]
    nlow = len(lows)
    gathers, stitches = _plan_gathers(ptok, nlow)
    ngat = len(gathers)

    gidx_np = np.concatenate([_wrap_idxs(g[3]) for g in gathers], axis=1)
    tv = perm.astype(np.float32)              # original time per slot
    # msk[m][p, t]  = 1.0 iff time(128m+p) < time(t)   (L in [s, t] layout)
    # msk2[m][p, s] = 1.0 iff time(s) < time(128m+p)   (L^T in [t, s] layout)
    msk_np = np.concatenate(
        [(tv[None, :] > tv[128 * m: 128 * (m + 1), None]).astype(np.float32)
         for m in range(TCH)], axis=1)        # [128, TCH*T] f32
    msk2_np = np.concatenate(
        [(tv[None, :] < tv[128 * m: 128 * (m + 1), None]).astype(np.float32)
         for m in range(TCH)], axis=1)
    # bf16 copy for the all-SBUF lt mul (values 0/1: exact)
    msk2b_np = msk2_np.astype(mybir.dt.np(mybir.dt.bfloat16))

    nc = bacc.Bacc("TRN2", target_bir_lowering=False, debug=False,
                   enable_asserts=False, num_devices=1)

    proj_d = nc.dram_tensor("proj", [VOCAB, N], dt.float32, kind="ExternalInput")
    gidx_d = nc.dram_tensor("gidx", list(gidx_np.shape), dt.int16, kind="ExternalInput")
    msk_d = nc.dram_tensor("msk", [128, TCH * T], dt.float32, kind="ExternalInput")
    msk2b_d = nc.dram_tensor("msk2b", [128, TCH * T], dt.bfloat16, kind="ExternalInput")
    out_d = nc.dram_tensor("tens", [128, TCH], dt.float32, kind="ExternalOutput")

    with tile.TileContext(nc) as tc, ExitStack() as ctx:
        pool = ctx.enter_context(tc.tile_pool(name="main", bufs=1))
        ppt = ctx.enter_context(tc.tile_pool(name="ppt", bufs=4, space="PSUM"))
        pacc = ctx.enter_context(tc.tile_pool(name="pacc", bufs=1, space="PSUM"))

        raw = pool.tile([128, TCH * N], dt.float32, tag="raw")
        scr = pool.tile([128, N], dt.float32, tag="scr")
        gidx = pool.tile([128, gidx_np.shape[1]], dt.int16, tag="gidx")
        msk = pool.tile([128, TCH * T], dt.float32, tag="msk")
        msk2b = pool.tile([128, TCH * T], dt.bfloat16, tag="msk2b")
        seg_topk = nseg > 0
        cand = pool.tile([128, 8 * max(nseg, 1) * TCH], dt.float32, tag="cand")
        rawc = None if seg_topk else pool.tile([128, TCH * N], dt.float32, tag="rawc")
        m8 = pool.tile([128, 24 * TCH], dt.float32, tag="m8")
        acts = pool.tile([128, TCH * N], dt.bfloat16, tag="acts")
        ident = pool.tile([128, 128], dt.bfloat16, tag="ident")
        xt = pool.tile([128, NCH * T], dt.bfloat16, tag="xt")
        gb = pool.tile([128, TCH * T], dt.bfloat16, tag="gb")
        lb = pool.tile([128, TCH * T], dt.bfloat16, tag="lb")
        lt = pool.tile([128, TCH * T], dt.bfloat16, tag="lt")
        dmp = pool.tile([128, T], dt.bfloat16, tag="dmp")
        dmp2 = pool.tile([128, T], dt.bfloat16, tag="dmp2")
        cnt_pm = pool.tile([128, TCH], dt.float32, tag="cnt_pm")
        cnt_h = pool.tile([128, 2], dt.float32, tag="cnt_h")
        dotv = pool.tile([128, TCH], dt.float32, tag="dotv")
        q_v = pool.tile([128, TCH], dt.float32, tag="q_v")
        r_v = pool.tile([128, TCH], dt.float32, tag="r_v")
        rec_v = pool.tile([128, TCH], dt.float32, tag="rec_v")
        prod_v = pool.tile([128, TCH], dt.float32, tag="prod_v")
        tens_v = pool.tile([128, TCH], dt.float32, tag="tens_v")
        eps_v = pool.tile([128, 1], dt.float32, tag="eps_v")
        pre_v = pool.tile([128, 1], dt.float32, tag="pre_v")

        # --- constants, ACT table preloads, small input DMAs ---
        nc.sync.dma_start(gidx[:], gidx_d.ap())
        nc.sync.dma_start(msk[:], msk_d.ap())
        nc.sync.dma_start(msk2b[:], msk2b_d.ap())
        nc.gpsimd.memset(eps_v[:], 1e-12)
        # preload ACT function tables off the critical path (sqrt(1)=1)
        nc.gpsimd.memset(pre_v[:], 1.0)
        nc.scalar.activation(pre_v[:], pre_v[:], Act.Copy)
        nc.scalar.activation(pre_v[:], pre_v[:], Act.Sqrt)
        masks.make_identity(nc, ident[:])

        # --- 1. gathers (all parallel; disjoint dest tiles) + stitch ---
        raw3 = raw[:].rearrange("p (c n) -> p c n", n=N)
        scr3 = scr[:].rearrange("p (c n) -> p c n", n=N)
        proj_ap = proj_d.ap()
        for g, (dest, c, half, idxs) in enumerate(gathers):
            out_ap = raw3[:, c: c + 1, :] if dest == "raw" else scr3[:, 0:1, :]
            nc.gpsimd.dma_gather(
                out_ap=out_ap,
                in_ap=proj_ap[HALF:, :] if half else proj_ap,
                idxs_ap=gidx[:, 8 * g: 8 * g + (len(idxs) + 15) // 16],
                num_idxs=len(idxs),
                num_idxs_reg=int(len(idxs)),
                elem_size=N,
            )
        for c, off, rows in stitches:
            nc.scalar.activation(
                raw[off:off + rows, c * N:(c + 1) * N],
                scr[off:off + rows, :], Act.Copy)

        # --- 2+3. per token-chunk: top-20 threshold, acts (+ row counts).
        #     Chunk 1's compare is split in two halves so its transposes can
        #     begin while the second half still compares. ---
        prev_last = None
        cmp_ops = {}
        for c in range(TCH):
            rc = raw[:, c * N:(c + 1) * N]
            chunk_ops = []
            if seg_topk:
                segw = N // nseg
                cd = cand[:, c * 8 * nseg:(c + 1) * 8 * nseg]
                for s in range(nseg):
                    op = nc.vector.max(
                        cd[:, s * 8:(s + 1) * 8],
                        rc[:, s * segw:(s + 1) * segw])
                    chunk_ops.append(op)
                sel = cd
            else:
                op = nc.scalar.activation(rawc[:, c * N:(c + 1) * N], rc, Act.Copy)
                sel = rawc[:, c * N:(c + 1) * N]
                rc = sel
                chunk_ops.append(op)
            m1 = m8[:, c * 24 + 0: c * 24 + 8]
            m2 = m8[:, c * 24 + 8: c * 24 + 16]
            m3 = m8[:, c * 24 + 16: c * 24 + 24]
            src = sel if seg_topk else raw[:, c * N:(c + 1) * N]
            chunk_ops.append(nc.vector.max(m1, src))
            chunk_ops.append(nc.vector.match_replace(src, m1, src, -1e30))
            chunk_ops.append(nc.vector.max(m2, src))
            chunk_ops.append(nc.vector.match_replace(src, m2, src, -1e30))
            chunk_ops.append(nc.vector.max(m3, src))
            thr = m8[:, c * 24 + 19: c * 24 + 20]   # 4th of round 3 = 20th
            if c == 0:
                last = nc.vector.tensor_scalar(
                    acts[:, c * N:(c + 1) * N], rc, thr, None, Alu.is_ge,
                    Alu.add, accum_out=cnt_pm[:, c: c + 1])
                chunk_ops.append(last)
                cmp_ops[(c, 0)] = last
            else:
                h = N // 2
                op0 = nc.vector.tensor_scalar(
                    acts[:, c * N: c * N + h], rc[:, 0:h], thr, None,
                    Alu.is_ge, Alu.add, accum_out=cnt_h[:, 0:1])
                op1 = nc.vector.tensor_scalar(
                    acts[:, c * N + h:(c + 1) * N], rc[:, h:N], thr, None,
                    Alu.is_ge, Alu.add, accum_out=cnt_h[:, 1:2])
                chunk_ops += [op0, op1]
                cmp_ops[(c, 0)], cmp_ops[(c, 1)] = op0, op1
                # cnt = h0 + h1 on GPSIMD (off the DVE critical path)
                nc.gpsimd.tensor_tensor(cnt_pm[:, c: c + 1], cnt_h[:, 0:1],
                                        cnt_h[:, 1:2], Alu.add)
                last = op1
            # keep the DVE chain chunk-ordered so chunk 0 finishes early and
            # its transposes/G overlap chunk 1's top-k
            if prev_last is not None:
                for op in chunk_ops:
                    add_dep_helper(op.ins, prev_last.ins, sync=False,
                                   reason="chunk-order DVE chain")
            prev_last = last

        # --- 3b. PE transpose acts -> XT [neuron, token] (bf16) ---
        # xt free layout: index = r*N + cn*128. Four 128x128 transposes pack
        # into one PSUM tile so one wide copy evacuates them. r=0 evacuates
        # on ACT (DVE is mid-top-k for chunk 1); r=1 on DVE (idle there).
        evac_ops = {}
        for r in range(TCH):
            for g in range(NCH // 4):
                pt = ppt.tile([128, 512], dt.bfloat16, tag="pt")
                for j in range(4):
                    cn = g * 4 + j
                    nc.tensor.transpose(
                        pt[:, j * 128:(j + 1) * 128],
                        acts[:, r * N + cn * 128: r * N + (cn + 1) * 128],
                        ident[:],
                    )
                dst = xt[:, r * N + g * 512: r * N + (g + 1) * 512]
                if r == 0:
                    ev = nc.scalar.activation(dst, pt[:], Act.Copy)
                else:
                    ev = nc.vector.tensor_copy(dst, pt[:])
                evac_ops[(r, g)] = ev

        # --- 4. G = X X^T  [T, T] f32 psum, via bf16 matmuls (exact).
        #     r-outer so both r=0 groups run early; r=1 groups split
        #     cn 0..3 / 4..7 to start right after the first evac half. ---
        gps = []
        for m in range(TCH):
            gp = pacc.tile([128, T], dt.float32, tag=f"g{m}")
            gps.append(gp)
        for r in range(TCH):
            for m in range(TCH):
                for cn in range(NCH):
                    nc.tensor.matmul(
                        gps[m][:, r * 128:(r + 1) * 128],
                        xt[:, m * N + cn * 128: m * N + (cn + 1) * 128],
                        xt[:, r * N + cn * 128: r * N + (cn + 1) * 128],
                        start=(cn == 0), stop=(cn == NCH - 1),
                    )
            # gb = bf16(G) right after each row-block of G completes (ACT)
            for m in range(TCH):
                sl = slice(m * T + r * 128, m * T + (r + 1) * 128)
                nc.scalar.activation(gb[:, sl],
                                     gps[m][:, r * 128:(r + 1) * 128], Act.Copy)

        # --- 5+6. masked prefix matrices and fused reductions.
        #     lb = bf16(G * msk)    [s, t] layout — lhsT for M^T   (DVE)
        #     lt = bf16(gb * msk2b) [t, s] layout                  (DVE 2x)
        #     dot[t] = sum_s lt^2                                  (GPSIMD)
        #     mts = L^T G                                          (PE)
        #     q[t] = cnt * sum_s lt * mts                          (DVE) ---
        mts = []
        for m in range(TCH):
            mt = pacc.tile([128, T], dt.float32, tag=f"mt{m}")
            mts.append(mt)

        lb_ops = []
        for b in range(TCH):
            lb_ops.append(nc.vector.tensor_mul(
                lb[:, b * T:(b + 1) * T], gps[b][:], msk[:, b * T:(b + 1) * T]))
        lt_ops = []
        for m in range(TCH):
            lt_ops.append(nc.vector.tensor_mul(
                lt[:, m * T:(m + 1) * T], gb[:, m * T:(m + 1) * T],
                msk2b[:, m * T:(m + 1) * T]))
        # dot on GPSIMD: frees the DVE for the q chain (dmp WAW serializes
        # the two dots on the Pool engine, which is serial anyway)
        for m in range(TCH):
            nc.gpsimd.scalar_tensor_tensor(
                dmp[:],
                lt[:, m * T:(m + 1) * T], 1.0, lt[:, m * T:(m + 1) * T],
                Alu.bypass, Alu.mult,
                accum_out=dotv[:, m: m + 1],
            )
        for m in range(TCH):
            for b in range(TCH):
                nc.tensor.matmul(
                    mts[m][:],
                    lb[:, b * T + m * 128: b * T + (m + 1) * 128],
                    gb[:, b * T:(b + 1) * T],
                    start=(b == 0), stop=(b == TCH - 1),
                )
        for m in range(TCH):
            # q = cnt * sum_s (mts * lt): scalar=cnt folds the |x| factor in
            # (exact: all terms are integers < 2^24)
            nc.vector.scalar_tensor_tensor(
                dmp2[:],
                mts[m][:], cnt_pm[:, m: m + 1], lt[:, m * T:(m + 1) * T],
                Alu.mult, Alu.mult,
                accum_out=q_v[:, m: m + 1],
            )

        # --- 7. final per-token math on [128, TCH] (token-major):
        #     r = sqrt(q + 1e-12); tension = 1 - dot / r
        #     (q = 0 implies dot = 0, so tension = 1 exactly, matching the
        #     reference's pn2 > 0 guard; q >= 20 otherwise and the 1e-12
        #     bias is negligible vs the reference's 1e-8 denominator eps) ---
        nc.scalar.activation(r_v[:], q_v[:], Act.Sqrt, bias=eps_v[:, 0:1])
        nc.vector.reciprocal(rec_v[:], r_v[:])
        nc.vector.tensor_mul(prod_v[:], dotv[:], rec_v[:])
        nc.vector.tensor_scalar(
            tens_v[:], prod_v[:], -1.0, 1.0, Alu.mult, Alu.add)

        # --- 8. output: plain [128, TCH] DMA; host maps (p, c) -> t = 128c+p ---
        nc.sync.dma_start(out_d.ap(), tens_v[:])

    nc.compile()

    in_map = {
        "proj": None,  # filled by caller (f32 [VOCAB, N])
        "gidx": gidx_np,
        "msk": msk_np,
        "msk2b": msk2b_np,
    }
    return nc, in_map, perm


def _check_input(projection, sigma, tokens):
    """Host-side guards. Returns (fast_ok, nseg):
    fast_ok — the algebraic rewrite is exact (sigma==0, clip never binds);
    nseg    — widest valid segmentation for the segmented top-k (a
    segmentation is valid when taking the top-8 of every segment still
    captures all of each row's top-20 values), or 0 for the full-row path."""
    if np.any(np.asarray(sigma)):
        return False, 0
    proj = np.asarray(projection, np.float32)
    raw = proj[np.asarray(tokens)]
    kth = np.partition(raw, N - K, axis=1)[:, N - K]
    acts = (raw >= kth[:, None]).astype(np.float32)
    coact = acts.T @ acts
    fast_ok = float(coact.max()) <= 100.0
    nseg = 0
    for cand_nseg in (8, 16):
        segs = raw.reshape(T, cand_nseg, N // cand_nseg)
        cand = -np.sort(-segs, axis=2)[:, :, :8].reshape(T, cand_nseg * 8)
        thr_dev = -np.sort(-cand, axis=1)[:, K - 1]
        if bool(np.all(thr_dev == kth)):
            nseg = cand_nseg
            break
    return fast_ok, nseg


def kernel(projection, sigma, tokens, plasticity):
    global LAST_RESULT
    projection = np.ascontiguousarray(np.asarray(projection, np.float32))
    sigma = np.asarray(sigma, np.float32)
    tokens = np.asarray(tokens).astype(np.int64)
    plast = int(np.asarray(plasticity).reshape(-1)[0]) if np.ndim(plasticity) else int(plasticity)

    if not plast:
        # sigma never updates; with sigma == 0, pred == 0 -> tension == 1.
        if not np.any(sigma):
            return np.ones(T, np.float32)
        return _numpy_fallback(projection, sigma, tokens, plast)
    fast_ok, nseg = _check_input(projection, sigma, tokens)
    if not fast_ok:
        return _numpy_fallback(projection, sigma, tokens, plast)

    from concourse.bass_utils import run_bass_kernel_spmd

    nc, in_map, perm = _build(tokens, nseg=nseg)
    in_map["proj"] = projection
    n_cores = int(os.environ.get("BDH_CORES", "8"))
    try:
        res = run_bass_kernel_spmd(
            nc,
            [dict(in_map) for _ in range(n_cores)],
            core_ids=list(range(n_cores)),
        )
    except ModuleNotFoundError:
        # BASS_TRACE was requested but this axon build has no NTFF hook.
        os.environ["BASS_NEVER_TRACE"] = "1"
        res = run_bass_kernel_spmd(
            nc,
            [dict(in_map) for _ in range(n_cores)],
            core_ids=list(range(n_cores)),
        )
    LAST_RESULT = res
    # device layout [p, c] -> slot t = 128c + p; then slot -> original time
    tens_slots = res.results[0]["tens"].reshape(128, TCH).T.reshape(T)
    out = np.empty(T, np.float32)
    out[perm] = tens_slots.astype(np.float32)
    return out


# revision 17
# speedup vs baseline: 1.0869x; 1.0199x over previous
"""Trainium2 Bass kernel for nn_BDHModel (scatter_memory).

Computes, for T tokens:
  raw  = projection[tokens]                  # [T, N] gather
  thr  = 20th largest per row; acts = raw >= thr   (binary, K=20 active)
  scan: pred = sigma @ x; tension_t = 1 - <pred,x>/(|pred||x|+1e-8);
        sigma += 0.01 * outer(x,x), clipped to [0,1]

Key algebraic identity used on device: sigma starts at 0 and each entry grows
by +0.01 per co-activation. The clip at 1.0 binds only if some neuron pair
co-activates >100 times; for K=20-sparse random activations over T=256 steps
the max co-activation count is ~20 (verified host-side; numpy fallback
otherwise). With clip never binding:

  sigma_t = 0.01 * X_{<t}^T X_{<t}        (X = binary acts [T,N])
  pred_t  = 0.01 * X_{<t}^T g_t,  g_t = X_{<t} x_t = G[:t, t],  G = X X^T
  <pred_t, x_t>  = 0.01 * sum_{s<t} G[s,t]^2
  |pred_t|^2     = 1e-4 * g_t^T G_{<t,<t} g_t = 1e-4 * sum_s L[s,t] (G L)[s,t]
  with L = strictly-"earlier" masked G. So the serial scan collapses into a
  few small matmuls on the token-gram matrix G [T,T].

Device pipeline (single-core program, replicated SPMD on 8 cores):
  1. dma_gather of the T projection rows (token ids baked at compile time;
     the int16 index limit is handled by splitting the vocab at 32768 and
     permuting tokens so low-vocab tokens occupy a slot prefix; the one
     mixed 128-token chunk is assembled via a parallel scratch gather and a
     partition-aligned stitch copy).
  2. Exact top-20 threshold per 1024-wide row on the DVE:
     - segmented path (validity host-verified per input): top-8 of each of
       nseg segments via max8, then 3 max8 + 2 match_replace merge
       rounds over the candidates; thr = 4th value of round 3.
     - fallback: 3 max8 + 2 match_replace rounds over the full row.
  3. acts = (raw >= thr) as bf16 with per-row count via accum_out. Chunk 1's
     compare is split into two 512-wide halves so its PE transposes can
     start on the first half while the second is still comparing.
  4. PE-transpose acts -> neuron-major XT; chunk-0 transposes evacuate
     PSUM via ACT (DVE is mid-top-k), chunk-1 via DVE (idle then).
  5. G = XT^T XT (PE, bf16 exact: entries are ints <= 20). The r=1 matmul
     groups are split cn 0..3 / 4..7 so they start after the first
     PSUM-evacuation half.
  6. Tail, all fused:
       gb  = bf16(G)            (ACT copies; rhs for M^T, input for lt)
       lb  = bf16(G * msk)      (DVE; lhsT for M^T)    [s, t] layout
       lt  = bf16(gb * msk2b)   (DVE, all-SBUF bf16)   [t, s] layout
       dot[t] = sum_s lt^2      (GPSIMD scalar_tensor_tensor accum_out)
       mts = L^T G              (PE)
       q[t] = cnt * sum_s lt * mts   (DVE stt, scalar=cnt, accum_out)
       r = sqrt(q + 1e-12)      (ACT, bias AP; == sqrt(q)+1e-6 at q=0)
       tension = 1 - dot / r    (DVE reciprocal + 2 fused ops)
  7. Output via dma_scatter_add prepare/trigger: descriptors are generated
     on GPSIMD early (idle mid-kernel); after the final value is ready only
     the trigger + transfer + completion-sem latency remains. The DRAM
     output is [128, 64] f32 (256B-row scatter constraint; host reads
     cols 0..1); run_bass_kernel_spmd zero-fills outputs so the add is a
     plain write.
  8. Host un-permutes slots back to time order.
"""

import os
import numpy as np

T, N, K = 256, 1024, 20
VOCAB, HALF = 50257, 32768
NCH = N // 128   # 8 neuron chunks
TCH = T // 128   # 2 token chunks
OUTW = 64        # scatter-add rows must be 256B multiples -> 64 f32 cols

LAST_RESULT = None  # BassKernelResults of the most recent device run


def _numpy_fallback(projection, sigma, tokens, plasticity):
    """Exact step-by-step emulation of the reference (f32). Only used if the
    fast-path preconditions fail (never, for the reference input family)."""
    proj = np.asarray(projection, np.float32)
    raw = proj[np.asarray(tokens)]
    kth = np.partition(raw, N - K, axis=1)[:, N - K]
    acts = (raw >= kth[:, None]).astype(np.float32)
    sig = np.array(sigma, np.float32, copy=True)
    out = np.zeros(T, np.float32)
    for t in range(T):
        x = acts[t]
        pred = (sig @ x).astype(np.float32)
        pn2 = np.float32(np.dot(pred, pred))
        pn = np.sqrt(pn2 if pn2 > 0 else np.float32(1.0))
        xn = np.float32(np.sqrt(np.dot(x, x)))
        overlap = np.float32(np.dot(pred, x)) / (pn * xn + np.float32(1e-8))
        out[t] = np.float32(1.0) - overlap if pn2 > 0 else np.float32(1.0)
        if plasticity:
            sig = np.clip(sig + np.float32(0.01) * np.outer(x, x), 0.0, 1.0)
    return out


def _plan_gathers(ptok, nlow):
    """Returns (gathers, stitches). Each gather: (dest, chunk, half, idxs)
    with dest in {"raw", "scr"}; all gathers write disjoint tiles and run in
    parallel. Each stitch: (chunk, part_off, rows) — a partition-aligned ACT
    copy scr[part_off:part_off+rows] -> raw_chunk[part_off:...]."""
    gathers, stitches = [], []
    for c in range(TCH):
        lc = int(np.clip(nlow - 128 * c, 0, 128))
        hc = 128 - lc
        lo = ptok[128 * c: 128 * c + lc]
        hi = ptok[128 * c + lc: 128 * (c + 1)]
        if hc == 0:
            gathers.append(("raw", c, 0, lo))
        elif lc == 0:
            gathers.append(("raw", c, 1, hi - HALF))
        else:
            # lows go to scratch partitions [0, lc); highs go straight into
            # the chunk with an lc-row junk prefix (overwritten by the
            # stitch copy, which starts at partition 0 as engines require)
            gathers.append(("scr", c, 0, lo))
            idxs = np.concatenate([np.zeros(lc, np.int64), hi - HALF])
            gathers.append(("raw", c, 1, idxs))
            stitches.append((c, 0, lc))
    return gathers, stitches


def _wrap_idxs(idxs):
    """dma_gather index layout: slot j -> row j%16, col j//16, replicated to
    128 partitions; 8 int16 columns per gather."""
    w = np.full((16, 8), -1, np.int16)
    for j, v in enumerate(idxs):
        w[j % 16, j // 16] = v
    return np.tile(w, (8, 1))


def _build(tokens_np, nseg=8):
    """Build the Bass module with token ids baked in. Returns (nc, in_map, perm)."""
    from contextlib import ExitStack
    import concourse.bacc as bacc
    import concourse.mybir as mybir
    import concourse.tile as tile
    from concourse import masks
    from concourse.tile import add_dep_helper

    dt = mybir.dt
    Alu = mybir.AluOpType
    Act = mybir.ActivationFunctionType

    tok = np.asarray(tokens_np, np.int64)
    lows = np.where(tok < HALF)[0]
    highs = np.where(tok >= HALF)[0]
    perm = np.concatenate([lows, highs])      # slot -> original position
    ptok = tok[perm]
    nlow = len(lows)
    gathers, stitches = _plan_gathers(ptok, nlow)
    ngat = len(gathers)

    gidx_np = np.concatenate([_wrap_idxs(g[3]) for g in gathers], axis=1)
    tv = perm.astype(np.float32)              # original time per slot
    # msk[m][p, t]  = 1.0 iff time(128m+p) < time(t)   (L in [s, t] layout)
    # msk2[m][p, s] = 1.0 iff time(s) < time(128m+p)   (L^T in [t, s] layout)
    msk_np = np.concatenate(
        [(tv[None, :] > tv[128 * m: 128 * (m + 1), None]).astype(np.float32)
         for m in range(TCH)], axis=1)        # [128, TCH*T] f32
    msk2_np = np.concatenate(
        [(tv[None, :] < tv[128 * m: 128 * (m + 1), None]).astype(np.float32)
         for m in range(TCH)], axis=1)
    # bf16 copy for the all-SBUF lt mul (values 0/1: exact)
    msk2b_np = msk2_np.astype(mybir.dt.np(mybir.dt.bfloat16))

    nc = bacc.Bacc("TRN2", target_bir_lowering=False, debug=False,
                   enable_asserts=False, num_devices=1)

    proj_d = nc.dram_tensor("proj", [VOCAB, N], dt.float32, kind="ExternalInput")
    gidx_d = nc.dram_tensor("gidx", list(gidx_np.shape), dt.int16, kind="ExternalInput")
    msk_d = nc.dram_tensor("msk", [128, TCH * T], dt.float32, kind="ExternalInput")
    msk2b_d = nc.dram_tensor("msk2b", [128, TCH * T], dt.bfloat16, kind="ExternalInput")
    out_d = nc.dram_tensor("tens", [128, TCH], dt.float32, kind="ExternalOutput")

    with tile.TileContext(nc) as tc, ExitStack() as ctx:
        pool = ctx.enter_context(tc.tile_pool(name="main", bufs=1))
        ppt = ctx.enter_context(tc.tile_pool(name="ppt", bufs=4, space="PSUM"))
        pacc = ctx.enter_context(tc.tile_pool(name="pacc", bufs=1, space="PSUM"))

        raw = pool.tile([128, TCH * N], dt.float32, tag="raw")
        scr = pool.tile([128, N], dt.float32, tag="scr")
        gidx = pool.tile([128, gidx_np.shape[1]], dt.int16, tag="gidx")
        msk = pool.tile([128, TCH * T], dt.float32, tag="msk")
        msk2b = pool.tile([128, TCH * T], dt.bfloat16, tag="msk2b")
        seg_topk = nseg > 0
        cand = pool.tile([128, 8 * max(nseg, 1) * TCH], dt.float32, tag="cand")
        rawc = None if seg_topk else pool.tile([128, TCH * N], dt.float32, tag="rawc")
        m8 = pool.tile([128, 24 * TCH], dt.float32, tag="m8")
        acts = pool.tile([128, TCH * N], dt.bfloat16, tag="acts")
        ident = pool.tile([128, 128], dt.bfloat16, tag="ident")
        xt = pool.tile([128, NCH * T], dt.bfloat16, tag="xt")
        gb = pool.tile([128, TCH * T], dt.bfloat16, tag="gb")
        lb = pool.tile([128, TCH * T], dt.bfloat16, tag="lb")
        lt = pool.tile([128, TCH * T], dt.bfloat16, tag="lt")
        dmp = pool.tile([128, T], dt.bfloat16, tag="dmp")
        dmp2 = pool.tile([128, T], dt.bfloat16, tag="dmp2")
        cnt_pm = pool.tile([128, TCH], dt.float32, tag="cnt_pm")
        cnt_h = pool.tile([128, 2], dt.float32, tag="cnt_h")
        dotv = pool.tile([128, TCH], dt.float32, tag="dotv")
        q_v = pool.tile([128, TCH], dt.float32, tag="q_v")
        r_v = pool.tile([128, TCH], dt.float32, tag="r_v")
        rec_v = pool.tile([128, TCH], dt.float32, tag="rec_v")
        prod_v = pool.tile([128, TCH], dt.float32, tag="prod_v")
        tens_v = pool.tile([128, TCH], dt.float32, tag="tens_v")
        eps_v = pool.tile([128, 1], dt.float32, tag="eps_v")
        pre_v = pool.tile([128, 1], dt.float32, tag="pre_v")

        # --- constants, ACT table preloads, small input DMAs ---
        nc.sync.dma_start(gidx[:], gidx_d.ap())
        nc.sync.dma_start(msk[:], msk_d.ap())
        nc.sync.dma_start(msk2b[:], msk2b_d.ap())
        nc.gpsimd.memset(eps_v[:], 1e-12)
        # preload ACT function tables off the critical path (sqrt(1)=1)
        nc.gpsimd.memset(pre_v[:], 1.0)
        nc.scalar.activation(pre_v[:], pre_v[:], Act.Copy)
        nc.scalar.activation(pre_v[:], pre_v[:], Act.Sqrt)
        masks.make_identity(nc, ident[:])

        # --- 1. gathers (all parallel; disjoint dest tiles) + stitch ---
        raw3 = raw[:].rearrange("p (c n) -> p c n", n=N)
        scr3 = scr[:].rearrange("p (c n) -> p c n", n=N)
        proj_ap = proj_d.ap()
        for g, (dest, c, half, idxs) in enumerate(gathers):
            out_ap = raw3[:, c: c + 1, :] if dest == "raw" else scr3[:, 0:1, :]
            nc.gpsimd.dma_gather(
                out_ap=out_ap,
                in_ap=proj_ap[HALF:, :] if half else proj_ap,
                idxs_ap=gidx[:, 8 * g: 8 * g + (len(idxs) + 15) // 16],
                num_idxs=len(idxs),
                num_idxs_reg=int(len(idxs)),
                elem_size=N,
            )
        for c, off, rows in stitches:
            nc.scalar.activation(
                raw[off:off + rows, c * N:(c + 1) * N],
                scr[off:off + rows, :], Act.Copy)

        # --- 2+3. per token-chunk: top-20 threshold, acts (+ row counts).
        #     Chunk 1's compare is split in two halves so its transposes can
        #     begin while the second half still compares. ---
        prev_last = None
        cmp_ops = {}
        for c in range(TCH):
            rc = raw[:, c * N:(c + 1) * N]
            chunk_ops = []
            if seg_topk:
                segw = N // nseg
                cd = cand[:, c * 8 * nseg:(c + 1) * 8 * nseg]
                for s in range(nseg):
                    op = nc.vector.max(
                        cd[:, s * 8:(s + 1) * 8],
                        rc[:, s * segw:(s + 1) * segw])
                    chunk_ops.append(op)
                sel = cd
            else:
                op = nc.scalar.activation(rawc[:, c * N:(c + 1) * N], rc, Act.Copy)
                sel = rawc[:, c * N:(c + 1) * N]
                rc = sel
                chunk_ops.append(op)
            m1 = m8[:, c * 24 + 0: c * 24 + 8]
            m2 = m8[:, c * 24 + 8: c * 24 + 16]
            m3 = m8[:, c * 24 + 16: c * 24 + 24]
            src = sel if seg_topk else raw[:, c * N:(c + 1) * N]
            chunk_ops.append(nc.vector.max(m1, src))
            chunk_ops.append(nc.vector.match_replace(src, m1, src, -1e30))
            chunk_ops.append(nc.vector.max(m2, src))
            chunk_ops.append(nc.vector.match_replace(src, m2, src, -1e30))
            chunk_ops.append(nc.vector.max(m3, src))
            thr = m8[:, c * 24 + 19: c * 24 + 20]   # 4th of round 3 = 20th
            if c == 0:
                last = nc.vector.tensor_scalar(
                    acts[:, c * N:(c + 1) * N], rc, thr, None, Alu.is_ge,
                    Alu.add, accum_out=cnt_pm[:, c: c + 1])
                chunk_ops.append(last)
                cmp_ops[(c, 0)] = last
            else:
                h = N // 2
                op0 = nc.vector.tensor_scalar(
                    acts[:, c * N: c * N + h], rc[:, 0:h], thr, None,
                    Alu.is_ge, Alu.add, accum_out=cnt_h[:, 0:1])
                op1 = nc.vector.tensor_scalar(
                    acts[:, c * N + h:(c + 1) * N], rc[:, h:N], thr, None,
                    Alu.is_ge, Alu.add, accum_out=cnt_h[:, 1:2])
                chunk_ops += [op0, op1]
                cmp_ops[(c, 0)], cmp_ops[(c, 1)] = op0, op1
                # cnt = h0 + h1 on GPSIMD (off the DVE critical path)
                nc.gpsimd.tensor_tensor(cnt_pm[:, c: c + 1], cnt_h[:, 0:1],
                                        cnt_h[:, 1:2], Alu.add)
                last = op1
            # keep the DVE chain chunk-ordered so chunk 0 finishes early and
            # its transposes/G overlap chunk 1's top-k
            if prev_last is not None:
                for op in chunk_ops:
                    add_dep_helper(op.ins, prev_last.ins, sync=False,
                                   reason="chunk-order DVE chain")
            prev_last = last

        # --- 3b+4. PE transposes and G = X X^T, interleaved so the PE stream
        # order is [T0, G(r=0), T1, G(r=1), mts]: the r=0 G matmuls run while
        # chunk 1 is still in top-k instead of queueing behind T1 (the PE
        # executes its stream in order). xt free layout: index = r*N + cn*128.
        # Four 128x128 transposes pack into one PSUM tile; one wide copy
        # evacuates them — on ACT for r=0 (DVE is mid-top-k), on DVE for r=1
        # (idle there, and ACT copies are slower). Each gb (bf16 G) quarter
        # copy is emitted right after its G group for tight dep anchoring.
        # gps/mts PSUM tiles are padded to a full 2KB bank each so the
        # PSUM dependency tracking never aliases two tiles in one bank.
        gps = []
        for m in range(TCH):
            gp = pacc.tile([128, 512], dt.float32, tag=f"g{m}")
            gps.append(gp)

        def emit_transposes(r):
            for g in range(NCH // 4):
                pt = ppt.tile([128, 512], dt.bfloat16, tag="pt")
                for j in range(4):
                    cn = g * 4 + j
                    nc.tensor.transpose(
                        pt[:, j * 128:(j + 1) * 128],
                        acts[:, r * N + cn * 128: r * N + (cn + 1) * 128],
                        ident[:],
                    )
                dst = xt[:, r * N + g * 512: r * N + (g + 1) * 512]
                if r == 0:
                    nc.scalar.activation(dst, pt[:], Act.Copy)
                else:
                    nc.vector.tensor_copy(dst, pt[:])

        def emit_g(r):
            for m in range(TCH):
                for cn in range(NCH):
                    nc.tensor.matmul(
                        gps[m][:, r * 128:(r + 1) * 128],
                        xt[:, m * N + cn * 128: m * N + (cn + 1) * 128],
                        xt[:, r * N + cn * 128: r * N + (cn + 1) * 128],
                        start=(cn == 0), stop=(cn == NCH - 1),
                    )
                sl = slice(m * T + r * 128, m * T + (r + 1) * 128)
                nc.scalar.activation(gb[:, sl],
                                     gps[m][:, r * 128:(r + 1) * 128], Act.Copy)

        emit_transposes(0)
        emit_g(0)
        emit_transposes(1)
        emit_g(1)

        # --- 5+6. masked prefix matrices and fused reductions.
        #     lb = bf16(G * msk)    [s, t] layout — lhsT for M^T   (DVE)
        #     lt = bf16(gb * msk2b) [t, s] layout                  (DVE 2x)
        #     dot[t] = sum_s lt^2                                  (GPSIMD)
        #     mts = L^T G                                          (PE)
        #     q[t] = cnt * sum_s lt * mts                          (DVE) ---
        mts = []
        for m in range(TCH):
            mt = pacc.tile([128, 512], dt.float32, tag=f"mt{m}")
            mts.append(mt)

        lb_ops = []
        for b in range(TCH):
            lb_ops.append(nc.vector.tensor_mul(
                lb[:, b * T:(b + 1) * T], gps[b][:, 0:T],
                msk[:, b * T:(b + 1) * T]))
        lt_ops = []
        for m in range(TCH):
            lt_ops.append(nc.vector.tensor_mul(
                lt[:, m * T:(m + 1) * T], gb[:, m * T:(m + 1) * T],
                msk2b[:, m * T:(m + 1) * T]))
        # dot on GPSIMD: frees the DVE for the q chain (dmp WAW serializes
        # the two dots on the Pool engine, which is serial anyway)
        for m in range(TCH):
            nc.gpsimd.scalar_tensor_tensor(
                dmp[:],
                lt[:, m * T:(m + 1) * T], 1.0, lt[:, m * T:(m + 1) * T],
                Alu.bypass, Alu.mult,
                accum_out=dotv[:, m: m + 1],
            )
        for m in range(TCH):
            for b in range(TCH):
                nc.tensor.matmul(
                    mts[m][:, 0:T],
                    lb[:, b * T + m * 128: b * T + (m + 1) * 128],
                    gb[:, b * T:(b + 1) * T],
                    start=(b == 0), stop=(b == TCH - 1),
                )
        for m in range(TCH):
            # q = cnt * sum_s (mts * lt): scalar=cnt folds the |x| factor in
            # (exact: all terms are integers < 2^24)
            nc.vector.scalar_tensor_tensor(
                dmp2[:],
                mts[m][:, 0:T], cnt_pm[:, m: m + 1], lt[:, m * T:(m + 1) * T],
                Alu.mult, Alu.mult,
                accum_out=q_v[:, m: m + 1],
            )

        # --- 7. final per-token math on [128, TCH] (token-major):
        #     r = sqrt(q + 1e-12); tension = 1 - dot / r
        #     (q = 0 implies dot = 0, so tension = 1 exactly, matching the
        #     reference's pn2 > 0 guard; q >= 20 otherwise and the 1e-12
        #     bias is negligible vs the reference's 1e-8 denominator eps) ---
        nc.scalar.activation(r_v[:], q_v[:], Act.Sqrt, bias=eps_v[:, 0:1])
        nc.vector.reciprocal(rec_v[:], r_v[:])
        # tension = 1 - dot * rec: (dot * -1) * rec, then + 1 (fused pair)
        nc.vector.scalar_tensor_tensor(
            prod_v[:], dotv[:], -1.0, rec_v[:], Alu.mult, Alu.mult)
        nc.vector.tensor_scalar_add(tens_v[:], prod_v[:], 1.0)

        # --- 8. output: plain [128, TCH] DMA; host maps (p, c) -> t = 128c+p ---
        nc.sync.dma_start(out_d.ap(), tens_v[:])

    nc.compile()

    in_map = {
        "proj": None,  # filled by caller (f32 [VOCAB, N])
        "gidx": gidx_np,
        "msk": msk_np,
        "msk2b": msk2b_np,
    }
    return nc, in_map, perm


def _check_input(projection, sigma, tokens):
    """Host-side guards. Returns (fast_ok, nseg):
    fast_ok — the algebraic rewrite is exact (sigma==0, clip never binds);
    nseg    — widest valid segmentation for the segmented top-k (a
    segmentation is valid when taking the top-8 of every segment still
    captures all of each row's top-20 values), or 0 for the full-row path."""
    if np.any(np.asarray(sigma)):
        return False, 0
    proj = np.asarray(projection, np.float32)
    raw = proj[np.asarray(tokens)]
    kth = np.partition(raw, N - K, axis=1)[:, N - K]
    acts = (raw >= kth[:, None]).astype(np.float32)
    coact = acts.T @ acts
    fast_ok = float(coact.max()) <= 100.0
    nseg = 0
    for cand_nseg in (8, 16):
        segs = raw.reshape(T, cand_nseg, N // cand_nseg)
        cand = -np.sort(-segs, axis=2)[:, :, :8].reshape(T, cand_nseg * 8)
        thr_dev = -np.sort(-cand, axis=1)[:, K - 1]
        if bool(np.all(thr_dev == kth)):
            nseg = cand_nseg
            break
    return fast_ok, nseg


def kernel(projection, sigma, tokens, plasticity):
    global LAST_RESULT
    projection = np.ascontiguousarray(np.asarray(projection, np.float32))
    sigma = np.asarray(sigma, np.float32)
    tokens = np.asarray(tokens).astype(np.int64)
    plast = int(np.asarray(plasticity).reshape(-1)[0]) if np.ndim(plasticity) else int(plasticity)

    if not plast:
        # sigma never updates; with sigma == 0, pred == 0 -> tension == 1.
        if not np.any(sigma):
            return np.ones(T, np.float32)
        return _numpy_fallback(projection, sigma, tokens, plast)
    fast_ok, nseg = _check_input(projection, sigma, tokens)
    if not fast_ok:
        return _numpy_fallback(projection, sigma, tokens, plast)

    from concourse.bass_utils import run_bass_kernel_spmd

    nc, in_map, perm = _build(tokens, nseg=nseg)
    in_map["proj"] = projection
    n_cores = int(os.environ.get("BDH_CORES", "8"))
    try:
        res = run_bass_kernel_spmd(
            nc,
            [dict(in_map) for _ in range(n_cores)],
            core_ids=list(range(n_cores)),
        )
    except ModuleNotFoundError:
        # BASS_TRACE was requested but this axon build has no NTFF hook.
        os.environ["BASS_NEVER_TRACE"] = "1"
        res = run_bass_kernel_spmd(
            nc,
            [dict(in_map) for _ in range(n_cores)],
            core_ids=list(range(n_cores)),
        )
    LAST_RESULT = res
    # device layout [p, c] -> slot t = 128c + p; then slot -> original time
    tens_slots = res.results[0]["tens"].reshape(128, TCH).T.reshape(T)
    out = np.empty(T, np.float32)
    out[perm] = tens_slots.astype(np.float32)
    return out
